# revision 48
# baseline (speedup 1.0000x reference)
"""Multi-head attention (12 heads, head_dim 64, RoPE, seq 1024) on 8 trn2 cores.

Sharding: pure data-parallel over the 16 (batch, row) units -> 2 per core.
No collectives. Each core runs the full per-unit attention.

One global software pipeline across both (b,r) units. The ACT
engine's exp stream (~200us/core) is the co-bottleneck with the PE
(~250us/core), so emission interleaves them:

  lead-in: x DMA prefetch (one strided DMA per unit -- each DMA issue
  costs ~600ns on the serial sync engine), xT, qk tiles 0-1 (so RoPE
  clears DVE's queue early), then v-proj of unit 0;
  then 24 attention iterations (2 units x 2 q-chunks x 6 head-pairs),
  each split into 8 per-kt segments emitting, in order:
    [scores kt]   2 scores matmuls (row groups alternate (0,0)/(64,0)
                  so adjacent pairs overlap in the PE array) + one
                  2-bank exp ACT into the shared [P,ST,2,QC] probs tile
    [fill unit]   (even kt) next chunk from a work queue: remaining qk
                  tiles of unit 0, then the whole projection phase of
                  unit 1; (odd kt>=5) out-projection chunks, which never
                  displace fills and sit late enough that the previous
                  normalize has cleared DVE's queue
    [pv kt]       PV+ones matmul col-pairs for the PREVIOUS iteration
  so the PE always has ready work while ACT drains the exp backlog.
  PSUM: shared proj/scores pool 2x2 banks + double-buffered 2-bank PV
  tile (exactly 8 banks) so no engine serializes on bank reuse.

  All matmul operands bf16 (FWL weight loads); psum fp32. QC=512.
  softmax: exp on ACT (scale=1/8, no max subtraction); sums broadcast
  across partitions by ones[128,64] matmuls into the same psum tile as
  PV; normalize = DVE reciprocal + TT mult. xT/v psum->sbuf copies run
  on the scalar engine (idle before/besides the exp stream) so DVE's
  in-order queue carries only bias+RoPE+normalize; the qk copies must
  NOT do this -- they'd queue behind the exp backlog and delay RoPE.

  biases: bq/bk applied in-kernel; bv/bo folded in on the host:
  out += bv @ Wo + bo (exact: sum(probs)=1).
  mask: all-ones fast path; any zero -> exact numpy fallback.
"""
from collections import deque

import numpy as np

H = 768
NH = 12
HD = 64
S = 1024
P = 128
DT = H // P          # 6 din/dout tiles
ST = S // P          # 8 seq tiles
BR = 2               # (b,r) units per core
NCORES = 8
QC = 512             # q-chunk
NQC = S // QC        # 2
ROPE_BASE = 10000.0

_CACHE = {}


def _rope_tables():
    inv = 1.0 / (ROPE_BASE ** (np.arange(0, HD, 2, dtype=np.float64) / HD))  # [32]
    t = np.arange(S, dtype=np.float64)
    f = np.outer(inv, t)                      # [32, S]
    cos2 = np.zeros((P, S), dtype=np.float32)
    sins = np.zeros((P, S), dtype=np.float32)
    c = np.cos(f).astype(np.float32)
    s = np.sin(f).astype(np.float32)
    for p in range(P):
        cos2[p] = c[p % 32]
        sins[p] = -s[p % 32] if (p % 64) < 32 else s[p % 32]
    return cos2, sins


def _legalize_waits(nc):
    """This walrus encodes at most one sync wait per instruction: hoist
    excess waits onto preceding same-engine NoOps."""
    import concourse.mybir as mybir

    n = 0
    for f in nc.m.functions:
        for blk in f.blocks:
            new = []
            for inst in blk.instructions:
                si = inst.sync_info
                waits = list(si.on_wait) if si and si.on_wait else []
                if len(waits) > 1:
                    for i, w in enumerate(waits[:-1]):
                        nop = mybir.InstNoOp(
                            name=f"{inst.name}-wn{i}", ins=[], outs=[],
                            sync_info=mybir.SyncInfo(on_wait=[w], on_update=[]))
                        nop.engine = inst.engine
                        new.append(nop)
                        n += 1
                    inst.sync_info = mybir.SyncInfo(
                        on_wait=[waits[-1]],
                        on_update=list(si.on_update) if si.on_update else [])
                new.append(inst)
            blk.instructions = new
    return n


def _build():
    import concourse.bass as bass
    import concourse.mybir as mybir
    import concourse.tile as tile

    F32 = mybir.dt.float32
    BF16 = mybir.dt.bfloat16
    Exp = mybir.ActivationFunctionType.Exp
    MUL = mybir.AluOpType.mult
    ADD = mybir.AluOpType.add

    nc = bass.Bass()
    xs = nc.dram_tensor("xs", [BR, S, H], BF16, kind="ExternalInput")
    wq = nc.dram_tensor("wq", [H, H], BF16, kind="ExternalInput")
    wk = nc.dram_tensor("wk", [H, H], BF16, kind="ExternalInput")
    wv = nc.dram_tensor("wv", [H, H], BF16, kind="ExternalInput")
    wo = nc.dram_tensor("wo", [H, H], BF16, kind="ExternalInput")
    bq = nc.dram_tensor("bq", [H], F32, kind="ExternalInput")
    bk = nc.dram_tensor("bk", [H], F32, kind="ExternalInput")
    cos2 = nc.dram_tensor("cos2", [P, S], BF16, kind="ExternalInput")
    sins = nc.dram_tensor("sins", [P, S], BF16, kind="ExternalInput")
    identm = nc.dram_tensor("identm", [P, P], BF16, kind="ExternalInput")
    onesb = nc.dram_tensor("onesb", [P, 64], BF16, kind="ExternalInput")
    out = nc.dram_tensor("out", [BR, S, H], F32, kind="ExternalOutput")

    with tile.TileContext(nc) as tc:
        with tc.tile_pool(name="const", bufs=1) as cpool, \
             tc.tile_pool(name="wpool", bufs=1) as wpool, \
             tc.tile_pool(name="xn", bufs=1) as xn_pool, \
             tc.tile_pool(name="xT", bufs=2) as xT_pool, \
             tc.tile_pool(name="qk", bufs=2) as qk_pool, \
             tc.tile_pool(name="rope", bufs=1) as rope_pool, \
             tc.tile_pool(name="vp", bufs=2) as v_pool, \
             tc.tile_pool(name="probs", bufs=1) as probs_pool, \
             tc.tile_pool(name="rec", bufs=1) as rec_pool, \
             tc.tile_pool(name="at", bufs=2) as at_pool, \
             tc.tile_pool(name="ot", bufs=2) as ot_pool, \
             tc.tile_pool(name="ps", bufs=2, space="PSUM") as ps_pool, \
             tc.tile_pool(name="pv", bufs=2, space="PSUM") as pv_pool:

            ident = cpool.tile([P, P], BF16, tag="ident")
            nc.sync.dma_start(ident[:], identm[:])

            # unit-0 x loads first so transposes can start ASAP; weights
            # and tables follow (not needed until v-proj, ~10us later)
            xn_tiles = {}

            def emit_xn_load(br):
                # one strided DMA for the whole unit: 8 separate loads cost
                # ~600ns EACH just to issue on the serial sync engine
                t = xn_pool.tile([P, ST, H], BF16, tag="xn", name="xn")
                nc.sync.dma_start(t[:], xs[br].rearrange("(t p) c -> p t c",
                                                         p=P))
                xn_tiles[br] = t

            emit_xn_load(0)

            w_sb = {}

            def load_w(name, w):
                w_sb[name] = wpool.tile([P, DT, H], BF16, tag=f"w{name}",
                                        name=f"w{name}")
                nc.sync.dma_start(
                    w_sb[name][:], w.rearrange("(t p) o -> p t o", p=P))

            load_w("q", wq)  # qk-proj runs right after the transposes
            load_w("k", wk)
            cos_sb = cpool.tile([P, S], BF16, tag="cos")
            sin_sb = cpool.tile([P, S], BF16, tag="sin")
            nc.sync.dma_start(cos_sb[:], cos2[:])
            nc.sync.dma_start(sin_sb[:], sins[:])
            bq_sb = cpool.tile([P, DT], F32, tag="bq")
            bk_sb = cpool.tile([P, DT], F32, tag="bk")
            nc.sync.dma_start(bq_sb[:], bq.rearrange("(t p) -> p t", p=P))
            nc.sync.dma_start(bk_sb[:], bk.rearrange("(t p) -> p t", p=P))
            load_w("v", wv)
            ones64 = cpool.tile([P, 64], BF16, tag="ones")
            nc.sync.dma_start(ones64[:], onesb[:])
            load_w("o", wo)

            # per-unit tile handles (pool tags rotate by allocation order)
            U = []
            for br in range(BR):
                xT = xT_pool.tile([P, DT, S], BF16, tag="xT", name=f"xT_{br}")
                v_sb = [v_pool.tile([P, H], BF16, tag=f"v{st}",
                                    name=f"v{st}_{br}") for st in range(ST)]
                qT = qk_pool.tile([P, DT, S], BF16, tag="qT", name=f"qT_{br}")
                kT = qk_pool.tile([P, DT, S], BF16, tag="kT", name=f"kT_{br}")
                U.append(dict(xT=xT, v=v_sb, q=qT, k=kT))

            bias_sb = {"q": bq_sb, "k": bk_sb}

            # warm the exp table set during the lead-in DMAs
            wup = cpool.tile([1, 4], F32, tag="wup")
            nc.scalar.activation(wup[:], bq_sb[0:1, 0:4],
                                 mybir.ActivationFunctionType.Exp)

            # ------------ emit helpers ------------
            def emit_xT_chunk(br, st):
                xT = U[br]["xT"]
                xn = xn_tiles[br]
                pt = ps_pool.tile([P, 1024], BF16, tag="ps", name="pt")
                for dj in range(DT):
                    nc.tensor.transpose(pt[:, dj * P:(dj + 1) * P],
                                        xn[:, st, dj * P:(dj + 1) * P], ident)
                # psum->sbuf copy on the (idle-at-this-point) scalar
                # engine so DVE's queue carries only bias+RoPE
                nc.scalar.copy(
                    xT[:, :, st * P:(st + 1) * P],
                    pt[:, 0:DT * P].rearrange("p (t c) -> p t c", c=P))

            def emit_v_chunk(br, st):
                xT, vt = U[br]["xT"], U[br]["v"][st]
                pp = ps_pool.tile([P, 2, 512], F32, tag="ps", name="pp")
                for nb in range(2):
                    c0 = nb * 384
                    for dj in range(DT):
                        nc.tensor.matmul(
                            pp[:, nb, 0:384],
                            xT[:, dj, st * P:(st + 1) * P],
                            w_sb["v"][:, dj, c0:c0 + 384],
                            start=(dj == 0), stop=(dj == DT - 1))
                nc.scalar.copy(
                    vt[:].rearrange("p (n c) -> p n c", n=2), pp[:, :, 0:384])

            def emit_qk_half(br, name, tt, half):
                xT, dst = U[br]["xT"], U[br][name]
                pp = ps_pool.tile([P, 512], F32, tag="ps", name="pp",
                                  uniquify=True)
                for dj in range(DT):
                    nc.tensor.matmul(
                        pp[:],
                        w_sb[name][:, dj, tt * P:(tt + 1) * P],
                        xT[:, dj, half * 512:(half + 1) * 512],
                        start=(dj == 0), stop=(dj == DT - 1))
                nc.vector.tensor_scalar_add(
                    dst[:, tt, half * 512:(half + 1) * 512], pp[:],
                    bias_sb[name][:, tt:tt + 1])
                if half == 1:
                    # RoPE: dst = dst*cos + swap(dst)*sins (on idle GpSimd so
                    # the qk->scores chain doesn't sit in DVE's in-order queue)
                    sw = rope_pool.tile([P, S], BF16, tag="ropesw", name="sw")
                    for hh2 in range(2):
                        b0 = hh2 * 64
                        nc.sync.dma_start(sw[b0:b0 + 32, :],
                                          dst[b0 + 32:b0 + 64, tt, :])
                        nc.sync.dma_start(sw[b0 + 32:b0 + 64, :],
                                          dst[b0:b0 + 32, tt, :])
                    nc.vector.tensor_tensor(sw[:], sw[:], sin_sb[:], MUL)
                    nc.vector.tensor_tensor(dst[:, tt, :], dst[:, tt, :],
                                            cos_sb[:], MUL)
                    nc.vector.tensor_tensor(dst[:, tt, :], dst[:, tt, :],
                                            sw[:], ADD)

            def emit_scores_kt(br, qc, hp, kt, prs):
                q0 = qc * QC
                qT, kT = U[br]["q"], U[br]["k"]
                sc = ps_pool.tile([P, 2, QC], F32, tag="ps", name="sc")
                for hh, base in ((0, 0), (1, 64)):
                    nc.tensor.matmul(
                        sc[:, hh, :],
                        kT[base:base + 64, hp, kt * P:(kt + 1) * P],
                        qT[base:base + 64, hp, q0:q0 + QC],
                        start=True, stop=True,
                        tile_position=(base, 0))
                nc.scalar.activation(prs[:, kt, :, :], sc[:], Exp, scale=0.125)

            def emit_pv_kt(br, hp, prs, pvt, kt):
                for hh in range(2):
                    nc.tensor.matmul(
                        pvt[hh * 64:(hh + 1) * 64, 0, :],
                        U[br]["v"][kt][:, (2 * hp + hh) * HD:
                                       (2 * hp + hh + 1) * HD],
                        prs[:, kt, hh, :],
                        start=(kt == 0), stop=(kt == ST - 1),
                        tile_position=(0, hh * 64))
                for hh in range(2):
                    nc.tensor.matmul(
                        pvt[hh * 64:(hh + 1) * 64, 1, :],
                        ones64[:], prs[:, kt, hh, :],
                        start=(kt == 0), stop=(kt == ST - 1),
                        tile_position=(0, hh * 64))

            def emit_normalize(hp, pvt, at):
                rec = rec_pool.tile([P, QC], F32, tag="rec", name="rec")
                nc.vector.reciprocal(rec[:], pvt[:, 1, :])
                att = at_pool.tile([P, QC], BF16, tag=f"at{hp}", name=f"at{hp}")
                at[hp] = att
                nc.vector.tensor_tensor(att[:], pvt[:, 0, :], rec[:], MUL)

            def emit_outproj_chunk(br, qc, at, sc2):
                q0 = qc * QC
                po = ps_pool.tile([P, 2, 512], F32, tag="ps", name="po")
                for nb in range(2):
                    c0 = nb * 384
                    for dj in range(DT):
                        nc.tensor.matmul(
                            po[:, nb, 0:384],
                            at[dj][:, sc2 * P:(sc2 + 1) * P],
                            w_sb["o"][:, dj, c0:c0 + 384],
                            start=(dj == 0), stop=(dj == DT - 1))
                ot = ot_pool.tile([P, H], F32, tag="ot", name="ot")
                nc.vector.tensor_copy(
                    ot[:].rearrange("p (n c) -> p n c", n=2), po[:, :, 0:384])
                r0 = q0 + sc2 * P
                nc.sync.dma_start(out[br, r0:r0 + P, :], ot[:])

            # ------- lead-in: unit 0 xT, qk tiles 0-1, then v -------
            # qk before v so RoPE (DVE+DMA) for the first scores completes
            # while the PE is still grinding v-proj matmuls
            for st in range(ST):
                emit_xT_chunk(0, st)
            for tt in range(2):
                for name in ("q", "k"):
                    for half in range(2):
                        emit_qk_half(0, name, tt, half)
            for st in range(ST):
                emit_v_chunk(0, st)
            emit_xn_load(1)

            # ------------ fill-unit work queue ------------
            fills = deque()
            for tt in range(2, DT):
                for name in ("q", "k"):
                    for half in range(2):
                        fills.append((emit_qk_half, (0, name, tt, half)))
            for st in range(ST):
                fills.append((emit_xT_chunk, (1, st)))
            for st in range(ST):
                fills.append((emit_v_chunk, (1, st)))
            for tt in range(DT):
                for name in ("q", "k"):
                    for half in range(2):
                        fills.append((emit_qk_half, (1, name, tt, half)))

            # ------------ global pipelined attention ------------
            iters = [(br, qc, hp) for br in range(BR)
                     for qc in range(NQC) for hp in range(DT)]
            at_cur = [{} for _ in range(BR)]
            prev = None
            pending_op = deque()   # out-projection chunks
            for (br, qc, hp) in iters:
                prs = probs_pool.tile([P, ST, 2, QC], BF16, tag="pr",
                                      name="pr", bufs=2)
                if prev is not None:
                    pvt = pv_pool.tile([P, 2, QC], F32, tag="pv", name="pvt")
                for kt in range(ST):
                    # scores first: ACT starts early in the segment, and the
                    # next psum-slot user (fill/scores) then waits on a fast
                    # DVE copy instead of a late exp ACT
                    emit_scores_kt(br, qc, hp, kt, prs)
                    if kt % 2 == 0:
                        if fills:
                            f, a = fills.popleft()
                            f(*a)
                    elif kt >= 5 and pending_op:
                        # out-proj chunks go in odd segments: they never
                        # displace fill units, and sit late enough that the
                        # previous normalize has cleared DVE's queue
                        f, a = pending_op.popleft()
                        f(*a)
                    if prev is not None:
                        emit_pv_kt(prev[0], prev[2], prev[3], pvt, kt)
                if prev is not None:
                    pbr, pqc, php, pprs = prev
                    emit_normalize(php, pvt, at_cur[pbr])
                    if php == DT - 1:
                        for sc2 in range(QC // P):
                            pending_op.append(
                                (emit_outproj_chunk,
                                 (pbr, pqc, dict(at_cur[pbr]), sc2)))
                prev = (br, qc, hp, prs)
            # drain: final pv + normalize + out-projection
            pbr, pqc, php, pprs = prev
            pvt = pv_pool.tile([P, 2, QC], F32, tag="pv", name="pvt")
            for kt in range(ST):
                if pending_op:
                    f, a = pending_op.popleft()
                    f(*a)
                emit_pv_kt(pbr, php, pprs, pvt, kt)
            emit_normalize(php, pvt, at_cur[pbr])
            while pending_op:
                f, a = pending_op.popleft()
                f(*a)
            for sc2 in range(QC // P):
                emit_outproj_chunk(pbr, pqc, at_cur[pbr], sc2)

    _legalize_waits(nc)
    return nc


def _get_nc():
    if "nc" not in _CACHE:
        _CACHE["nc"] = _build()
    return _CACHE["nc"]


def _numpy_reference(x, Wq, bq, Wk, bk, Wv, bv, Wo, bo, mask):
    b, r, s, d = x.shape
    inv = 1.0 / (ROPE_BASE ** (np.arange(0, HD, 2, dtype=np.float32) / HD))
    t = np.arange(s, dtype=np.float32)
    f = np.outer(t, inv)
    emb = np.concatenate([f, f], axis=-1)
    cos, sin = np.cos(emb), np.sin(emb)

    def proj(W, bvec):
        y = x @ W + bvec
        return y.reshape(b, r, s, NH, HD).transpose(0, 1, 3, 2, 4)

    def rot(z):
        z1, z2 = z[..., :HD // 2], z[..., HD // 2:]
        return np.concatenate([-z2, z1], axis=-1)

    q = proj(Wq, bq)
    k = proj(Wk, bk)
    v = proj(Wv, bv)
    q = q * cos + rot(q) * sin
    k = k * cos + rot(k) * sin
    scores = np.einsum("brhqd,brhkd->brhqk", q, k) / np.sqrt(np.float32(HD))
    scores = np.where(mask == 0, -np.inf, scores)
    m = scores.max(axis=-1, keepdims=True)
    e = np.exp(scores - m)
    probs = e / e.sum(axis=-1, keepdims=True)
    o = np.einsum("brhqk,brhkd->brhqd", probs, v)
    o = o.transpose(0, 1, 3, 2, 4).reshape(b, r, s, d)
    return (o @ Wo + bo).astype(np.float32)


def _run(inputs, trace=False):
    import ml_dtypes
    from concourse.bass_utils import run_bass_kernel_spmd

    BF = ml_dtypes.bfloat16
    x = np.asarray(inputs["x"], dtype=np.float32)
    Wq = np.ascontiguousarray(np.asarray(inputs["Wq"], dtype=np.float32))
    Wk = np.ascontiguousarray(np.asarray(inputs["Wk"], dtype=np.float32))
    Wv = np.ascontiguousarray(np.asarray(inputs["Wv"], dtype=np.float32))
    Wo = np.ascontiguousarray(np.asarray(inputs["Wo"], dtype=np.float32))
    bq = np.asarray(inputs["bq"], dtype=np.float32)
    bk = np.asarray(inputs["bk"], dtype=np.float32)
    bv = np.asarray(inputs["bv"], dtype=np.float32)
    bo = np.asarray(inputs["bo"], dtype=np.float32)

    xf = np.ascontiguousarray(x.reshape(NCORES * BR, S, H).astype(BF))
    cos2, sins = _rope_tables()
    cos2 = cos2.astype(BF)
    sins = sins.astype(BF)
    identm = np.eye(P, dtype=np.float32).astype(BF)
    onesb = np.ones((P, 64), dtype=BF)
    wqb, wkb, wvb, wob = (np.ascontiguousarray(w.astype(BF))
                          for w in (Wq, Wk, Wv, Wo))
    nc = _get_nc()
    in_maps = []
    for c in range(NCORES):
        in_maps.append(dict(
            xs=np.ascontiguousarray(xf[c * BR:(c + 1) * BR]),
            wq=wqb, wk=wkb, wv=wvb, wo=wob, bq=bq, bk=bk,
            cos2=cos2, sins=sins, identm=identm, onesb=onesb))
    res = run_bass_kernel_spmd(nc, in_maps, core_ids=list(range(NCORES)),
                               trace=trace)
    outs = np.concatenate([r["out"] for r in res.results], axis=0)
    out = outs.reshape(2, NCORES * BR // 2, S, H)
    out = out + (bv @ Wo + bo)
    return out.astype(np.float32), res


def kernel(**inputs):
    mask = np.asarray(inputs["mask"])
    if not np.all(mask != 0):
        return _numpy_reference(
            x=np.asarray(inputs["x"], np.float32),
            Wq=np.asarray(inputs["Wq"], np.float32),
            bq=np.asarray(inputs["bq"], np.float32),
            Wk=np.asarray(inputs["Wk"], np.float32),
            bk=np.asarray(inputs["bk"], np.float32),
            Wv=np.asarray(inputs["Wv"], np.float32),
            bv=np.asarray(inputs["bv"], np.float32),
            Wo=np.asarray(inputs["Wo"], np.float32),
            bo=np.asarray(inputs["bo"], np.float32),
            mask=mask)
    out, _ = _run(inputs, trace=False)
    return out


# revision 50
# speedup vs baseline: 1.0057x; 1.0057x over previous
"""Multi-head attention (12 heads, head_dim 64, RoPE, seq 1024) on 8 trn2 cores.

Sharding: pure data-parallel over the 16 (batch, row) units -> 2 per core.
No collectives. Each core runs the full per-unit attention.

One global software pipeline across both (b,r) units. The ACT
engine's exp stream (~200us/core) is the co-bottleneck with the PE
(~250us/core), so emission interleaves them:

  lead-in: x DMA prefetch (one strided DMA per unit -- each DMA issue
  costs ~600ns on the serial sync engine), xT, qk tiles 0-1 (so RoPE
  clears DVE's queue early), then v-proj of unit 0;
  then 24 attention iterations (2 units x 2 q-chunks x 6 head-pairs),
  each split into 8 per-kt segments emitting, in order:
    [scores kt]   2 scores matmuls (row groups alternate (0,0)/(64,0)
                  so adjacent pairs overlap in the PE array) + one
                  2-bank exp ACT into the shared [P,ST,2,QC] probs tile
    [fill unit]   (even kt) next chunk from a work queue: remaining qk
                  tiles of unit 0, then the whole projection phase of
                  unit 1; (odd kt>=5) out-projection chunks, which never
                  displace fills and sit late enough that the previous
                  normalize has cleared DVE's queue
    [pv kt]       PV+ones matmul col-pairs for the PREVIOUS iteration
  so the PE always has ready work while ACT drains the exp backlog.
  PSUM: shared proj/scores pool 2x2 banks + double-buffered 2-bank PV
  tile (exactly 8 banks) so no engine serializes on bank reuse.

  All matmul operands bf16 (FWL weight loads); psum fp32. QC=512.
  softmax: exp on ACT (scale=1/8, no max subtraction); sums broadcast
  across partitions by ones[128,64] matmuls into the same psum tile as
  PV; normalize = DVE reciprocal + TT mult. xT/v psum->sbuf copies run
  on the scalar engine (idle before/besides the exp stream) so DVE's
  in-order queue carries only bias+RoPE+normalize; the qk copies must
  NOT do this -- they'd queue behind the exp backlog and delay RoPE.

  biases: bq/bk applied in-kernel; bv/bo folded in on the host:
  out += bv @ Wo + bo (exact: sum(probs)=1).
  mask: all-ones fast path; any zero -> exact numpy fallback.
"""
from collections import deque

import numpy as np

H = 768
NH = 12
HD = 64
S = 1024
P = 128
DT = H // P          # 6 din/dout tiles
ST = S // P          # 8 seq tiles
BR = 2               # (b,r) units per core
NCORES = 8
QC = 512             # q-chunk
NQC = S // QC        # 2
ROPE_BASE = 10000.0

_CACHE = {}


def _rope_tables():
    inv = 1.0 / (ROPE_BASE ** (np.arange(0, HD, 2, dtype=np.float64) / HD))  # [32]
    t = np.arange(S, dtype=np.float64)
    f = np.outer(inv, t)                      # [32, S]
    cos2 = np.zeros((P, S), dtype=np.float32)
    sins = np.zeros((P, S), dtype=np.float32)
    c = np.cos(f).astype(np.float32)
    s = np.sin(f).astype(np.float32)
    for p in range(P):
        cos2[p] = c[p % 32]
        sins[p] = -s[p % 32] if (p % 64) < 32 else s[p % 32]
    return cos2, sins


def _legalize_waits(nc):
    """This walrus encodes at most one sync wait per instruction: hoist
    excess waits onto preceding same-engine NoOps."""
    import concourse.mybir as mybir

    n = 0
    for f in nc.m.functions:
        for blk in f.blocks:
            new = []
            for inst in blk.instructions:
                si = inst.sync_info
                waits = list(si.on_wait) if si and si.on_wait else []
                if len(waits) > 1:
                    for i, w in enumerate(waits[:-1]):
                        nop = mybir.InstNoOp(
                            name=f"{inst.name}-wn{i}", ins=[], outs=[],
                            sync_info=mybir.SyncInfo(on_wait=[w], on_update=[]))
                        nop.engine = inst.engine
                        new.append(nop)
                        n += 1
                    inst.sync_info = mybir.SyncInfo(
                        on_wait=[waits[-1]],
                        on_update=list(si.on_update) if si.on_update else [])
                new.append(inst)
            blk.instructions = new
    return n


def _build():
    import concourse.bass as bass
    import concourse.mybir as mybir
    import concourse.tile as tile

    F32 = mybir.dt.float32
    BF16 = mybir.dt.bfloat16
    Exp = mybir.ActivationFunctionType.Exp
    MUL = mybir.AluOpType.mult
    ADD = mybir.AluOpType.add

    nc = bass.Bass()
    xs = nc.dram_tensor("xs", [BR, S, H], BF16, kind="ExternalInput")
    wq = nc.dram_tensor("wq", [H, H], BF16, kind="ExternalInput")
    wk = nc.dram_tensor("wk", [H, H], BF16, kind="ExternalInput")
    wv = nc.dram_tensor("wv", [H, H], BF16, kind="ExternalInput")
    wo = nc.dram_tensor("wo", [H, H], BF16, kind="ExternalInput")
    bq = nc.dram_tensor("bq", [H], F32, kind="ExternalInput")
    bk = nc.dram_tensor("bk", [H], F32, kind="ExternalInput")
    cos2 = nc.dram_tensor("cos2", [P, S], BF16, kind="ExternalInput")
    sins = nc.dram_tensor("sins", [P, S], BF16, kind="ExternalInput")
    identm = nc.dram_tensor("identm", [P, P], BF16, kind="ExternalInput")
    onesb = nc.dram_tensor("onesb", [P, 64], BF16, kind="ExternalInput")
    out = nc.dram_tensor("out", [BR, S, H], F32, kind="ExternalOutput")

    with tile.TileContext(nc) as tc:
        with tc.tile_pool(name="const", bufs=1) as cpool, \
             tc.tile_pool(name="wpool", bufs=1) as wpool, \
             tc.tile_pool(name="xn", bufs=1) as xn_pool, \
             tc.tile_pool(name="xT", bufs=2) as xT_pool, \
             tc.tile_pool(name="qk", bufs=2) as qk_pool, \
             tc.tile_pool(name="rope", bufs=1) as rope_pool, \
             tc.tile_pool(name="vp", bufs=2) as v_pool, \
             tc.tile_pool(name="probs", bufs=1) as probs_pool, \
             tc.tile_pool(name="rec", bufs=1) as rec_pool, \
             tc.tile_pool(name="at", bufs=2) as at_pool, \
             tc.tile_pool(name="ot", bufs=2) as ot_pool, \
             tc.tile_pool(name="ps", bufs=2, space="PSUM") as ps_pool, \
             tc.tile_pool(name="pv", bufs=2, space="PSUM") as pv_pool:

            ident = cpool.tile([P, P], BF16, tag="ident")
            nc.sync.dma_start(ident[:], identm[:])

            # unit-0 x loads first so transposes can start ASAP; weights
            # and tables follow (not needed until v-proj, ~10us later)
            xn_tiles = {}

            def emit_xn_load(br):
                # one strided DMA for the whole unit: 8 separate loads cost
                # ~600ns EACH just to issue on the serial sync engine
                t = xn_pool.tile([P, ST, H], BF16, tag="xn", name="xn")
                nc.sync.dma_start(t[:], xs[br].rearrange("(t p) c -> p t c",
                                                         p=P))
                xn_tiles[br] = t

            emit_xn_load(0)

            w_sb = {}

            def load_w(name, w):
                w_sb[name] = wpool.tile([P, DT, H], BF16, tag=f"w{name}",
                                        name=f"w{name}")
                nc.sync.dma_start(
                    w_sb[name][:], w.rearrange("(t p) o -> p t o", p=P))

            load_w("q", wq)  # qk-proj runs right after the transposes
            load_w("k", wk)
            cos_sb = cpool.tile([P, S], BF16, tag="cos")
            sin_sb = cpool.tile([P, S], BF16, tag="sin")
            nc.sync.dma_start(cos_sb[:], cos2[:])
            nc.sync.dma_start(sin_sb[:], sins[:])
            bq_sb = cpool.tile([P, DT], F32, tag="bq")
            bk_sb = cpool.tile([P, DT], F32, tag="bk")
            nc.sync.dma_start(bq_sb[:], bq.rearrange("(t p) -> p t", p=P))
            nc.sync.dma_start(bk_sb[:], bk.rearrange("(t p) -> p t", p=P))
            load_w("v", wv)
            ones64 = cpool.tile([P, 64], BF16, tag="ones")
            nc.sync.dma_start(ones64[:], onesb[:])
            load_w("o", wo)

            # per-unit tile handles (pool tags rotate by allocation order)
            U = []
            for br in range(BR):
                xT = xT_pool.tile([P, DT, S], BF16, tag="xT", name=f"xT_{br}")
                v_sb = [v_pool.tile([P, H], BF16, tag=f"v{st}",
                                    name=f"v{st}_{br}") for st in range(ST)]
                qT = qk_pool.tile([P, DT, S], BF16, tag="qT", name=f"qT_{br}")
                kT = qk_pool.tile([P, DT, S], BF16, tag="kT", name=f"kT_{br}")
                U.append(dict(xT=xT, v=v_sb, q=qT, k=kT))

            bias_sb = {"q": bq_sb, "k": bk_sb}

            # warm the exp table set during the lead-in DMAs
            wup = cpool.tile([1, 4], F32, tag="wup")
            nc.scalar.activation(wup[:], bq_sb[0:1, 0:4],
                                 mybir.ActivationFunctionType.Exp)

            # ------------ emit helpers ------------
            def emit_xT_chunk(br, st):
                xT = U[br]["xT"]
                xn = xn_tiles[br]
                pt = ps_pool.tile([P, 1024], BF16, tag="ps", name="pt")
                for dj in range(DT):
                    nc.tensor.transpose(pt[:, dj * P:(dj + 1) * P],
                                        xn[:, st, dj * P:(dj + 1) * P], ident)
                # psum->sbuf copy on the (idle-at-this-point) scalar
                # engine so DVE's queue carries only bias+RoPE
                nc.scalar.copy(
                    xT[:, :, st * P:(st + 1) * P],
                    pt[:, 0:DT * P].rearrange("p (t c) -> p t c", c=P))

            def emit_v_chunk(br, st):
                xT, vt = U[br]["xT"], U[br]["v"][st]
                pp = ps_pool.tile([P, 2, 512], F32, tag="ps", name="pp")
                for nb in range(2):
                    c0 = nb * 384
                    for dj in range(DT):
                        nc.tensor.matmul(
                            pp[:, nb, 0:384],
                            xT[:, dj, st * P:(st + 1) * P],
                            w_sb["v"][:, dj, c0:c0 + 384],
                            start=(dj == 0), stop=(dj == DT - 1))
                nc.scalar.copy(
                    vt[:].rearrange("p (n c) -> p n c", n=2), pp[:, :, 0:384])

            def emit_qk_half(br, name, tt, half):
                xT, dst = U[br]["xT"], U[br][name]
                pp = ps_pool.tile([P, 512], F32, tag="ps", name="pp",
                                  uniquify=True)
                for dj in range(DT):
                    nc.tensor.matmul(
                        pp[:],
                        w_sb[name][:, dj, tt * P:(tt + 1) * P],
                        xT[:, dj, half * 512:(half + 1) * 512],
                        start=(dj == 0), stop=(dj == DT - 1))
                nc.vector.tensor_scalar_add(
                    dst[:, tt, half * 512:(half + 1) * 512], pp[:],
                    bias_sb[name][:, tt:tt + 1])
                if half == 1:
                    # RoPE: dst = dst*cos + swap(dst)*sins (on idle GpSimd so
                    # the qk->scores chain doesn't sit in DVE's in-order queue)
                    sw = rope_pool.tile([P, S], BF16, tag="ropesw", name="sw")
                    for hh2 in range(2):
                        b0 = hh2 * 64
                        nc.sync.dma_start(sw[b0:b0 + 32, :],
                                          dst[b0 + 32:b0 + 64, tt, :])
                        nc.sync.dma_start(sw[b0 + 32:b0 + 64, :],
                                          dst[b0:b0 + 32, tt, :])
                    nc.vector.tensor_tensor(sw[:], sw[:], sin_sb[:], MUL)
                    nc.vector.tensor_tensor(dst[:, tt, :], dst[:, tt, :],
                                            cos_sb[:], MUL)
                    nc.vector.tensor_tensor(dst[:, tt, :], dst[:, tt, :],
                                            sw[:], ADD)

            def emit_scores_kt(br, qc, hp, kt, prs):
                q0 = qc * QC
                qT, kT = U[br]["q"], U[br]["k"]
                sc = ps_pool.tile([P, 2, QC], F32, tag="ps", name="sc")
                for hh, base in ((0, 0), (1, 64)):
                    nc.tensor.matmul(
                        sc[:, hh, :],
                        kT[base:base + 64, hp, kt * P:(kt + 1) * P],
                        qT[base:base + 64, hp, q0:q0 + QC],
                        start=True, stop=True,
                        tile_position=(base, 0))
                nc.scalar.activation(prs[:, kt, :, :], sc[:], Exp, scale=0.125)

            def emit_pv_kt(br, hp, prs, pvt, kt):
                for hh in range(2):
                    nc.tensor.matmul(
                        pvt[hh * 64:(hh + 1) * 64, 0, :],
                        U[br]["v"][kt][:, (2 * hp + hh) * HD:
                                       (2 * hp + hh + 1) * HD],
                        prs[:, kt, hh, :],
                        start=(kt == 0), stop=(kt == ST - 1),
                        tile_position=(0, hh * 64))
                for hh in range(2):
                    nc.tensor.matmul(
                        pvt[hh * 64:(hh + 1) * 64, 1, :],
                        ones64[:], prs[:, kt, hh, :],
                        start=(kt == 0), stop=(kt == ST - 1),
                        tile_position=(0, hh * 64))

            def emit_normalize(hp, pvt, at):
                rec = rec_pool.tile([P, QC], F32, tag="rec", name="rec")
                nc.vector.reciprocal(rec[:], pvt[:, 1, :])
                att = at_pool.tile([P, QC], BF16, tag=f"at{hp}", name=f"at{hp}")
                at[hp] = att
                nc.vector.tensor_tensor(att[:], pvt[:, 0, :], rec[:], MUL)

            def emit_outproj_chunk(br, qc, at, sc2):
                q0 = qc * QC
                po = ps_pool.tile([P, 2, 512], F32, tag="ps", name="po")
                for nb in range(2):
                    c0 = nb * 384
                    for dj in range(DT):
                        nc.tensor.matmul(
                            po[:, nb, 0:384],
                            at[dj][:, sc2 * P:(sc2 + 1) * P],
                            w_sb["o"][:, dj, c0:c0 + 384],
                            start=(dj == 0), stop=(dj == DT - 1))
                ot = ot_pool.tile([P, H], F32, tag="ot", name="ot")
                nc.vector.tensor_copy(
                    ot[:].rearrange("p (n c) -> p n c", n=2), po[:, :, 0:384])
                r0 = q0 + sc2 * P
                nc.sync.dma_start(out[br, r0:r0 + P, :], ot[:])

            # ------- lead-in: unit 0 xT, qk tiles 0-1, then v -------
            # qk before v so RoPE (DVE+DMA) for the first scores completes
            # while the PE is still grinding v-proj matmuls
            for st in range(ST):
                emit_xT_chunk(0, st)
            for tt in range(2):
                for name in ("q", "k"):
                    for half in range(2):
                        emit_qk_half(0, name, tt, half)
            for st in range(ST):
                emit_v_chunk(0, st)
            emit_xn_load(1)

            # ------------ fill-unit work queue ------------
            fills = deque()
            for tt in range(2, DT):
                for name in ("q", "k"):
                    for half in range(2):
                        fills.append((emit_qk_half, (0, name, tt, half)))
            for st in range(ST):
                fills.append((emit_xT_chunk, (1, st)))
            for st in range(ST):
                fills.append((emit_v_chunk, (1, st)))
            for tt in range(DT):
                for name in ("q", "k"):
                    for half in range(2):
                        fills.append((emit_qk_half, (1, name, tt, half)))

            # ------------ global pipelined attention ------------
            iters = [(br, qc, hp) for br in range(BR)
                     for qc in range(NQC) for hp in range(DT)]
            at_cur = [{} for _ in range(BR)]
            prev = None
            pending_op = deque()   # out-projection chunks
            for (br, qc, hp) in iters:
                prs = probs_pool.tile([P, ST, 2, QC], BF16, tag="pr",
                                      name="pr", bufs=2)
                if prev is not None:
                    pvt = pv_pool.tile([P, 2, QC], F32, tag="pv", name="pvt")
                for kt in range(ST):
                    # scores first: ACT starts early in the segment, and the
                    # next psum-slot user (fill/scores) then waits on a fast
                    # DVE copy instead of a late exp ACT
                    emit_scores_kt(br, qc, hp, kt, prs)
                    if kt % 2 == 0:
                        if fills:
                            f, a = fills.popleft()
                            f(*a)
                    elif kt >= 5 and pending_op:
                        # out-proj chunks go in odd segments: they never
                        # displace fill units, and sit late enough that the
                        # previous normalize has cleared DVE's queue
                        f, a = pending_op.popleft()
                        f(*a)
                    if prev is not None:
                        emit_pv_kt(prev[0], prev[2], prev[3], pvt, kt)
                if prev is not None:
                    pbr, pqc, php, pprs = prev
                    emit_normalize(php, pvt, at_cur[pbr])
                    if php == DT - 1:
                        for sc2 in range(QC // P):
                            pending_op.append(
                                (emit_outproj_chunk,
                                 (pbr, pqc, dict(at_cur[pbr]), sc2)))
                prev = (br, qc, hp, prs)
            # drain: final pv + normalize + out-projection
            pbr, pqc, php, pprs = prev
            pvt = pv_pool.tile([P, 2, QC], F32, tag="pv", name="pvt")
            for kt in range(ST):
                if pending_op:
                    f, a = pending_op.popleft()
                    f(*a)
                emit_pv_kt(pbr, php, pprs, pvt, kt)
            emit_normalize(php, pvt, at_cur[pbr])
            while pending_op:
                f, a = pending_op.popleft()
                f(*a)
            for sc2 in range(QC // P):
                emit_outproj_chunk(pbr, pqc, at_cur[pbr], sc2)

    _legalize_waits(nc)
    return nc


def _get_nc():
    if "nc" not in _CACHE:
        _CACHE["nc"] = _build()
    return _CACHE["nc"]


def _numpy_reference(x, Wq, bq, Wk, bk, Wv, bv, Wo, bo, mask):
    b, r, s, d = x.shape
    inv = 1.0 / (ROPE_BASE ** (np.arange(0, HD, 2, dtype=np.float32) / HD))
    t = np.arange(s, dtype=np.float32)
    f = np.outer(t, inv)
    emb = np.concatenate([f, f], axis=-1)
    cos, sin = np.cos(emb), np.sin(emb)

    def proj(W, bvec):
        y = x @ W + bvec
        return y.reshape(b, r, s, NH, HD).transpose(0, 1, 3, 2, 4)

    def rot(z):
        z1, z2 = z[..., :HD // 2], z[..., HD // 2:]
        return np.concatenate([-z2, z1], axis=-1)

    q = proj(Wq, bq)
    k = proj(Wk, bk)
    v = proj(Wv, bv)
    q = q * cos + rot(q) * sin
    k = k * cos + rot(k) * sin
    scores = np.einsum("brhqd,brhkd->brhqk", q, k) / np.sqrt(np.float32(HD))
    scores = np.where(mask == 0, -np.inf, scores)
    m = scores.max(axis=-1, keepdims=True)
    e = np.exp(scores - m)
    probs = e / e.sum(axis=-1, keepdims=True)
    o = np.einsum("brhqk,brhkd->brhqd", probs, v)
    o = o.transpose(0, 1, 3, 2, 4).reshape(b, r, s, d)
    return (o @ Wo + bo).astype(np.float32)


def _run(inputs, trace=False):
    import ml_dtypes
    from concourse.bass_utils import run_bass_kernel_spmd

    BF = ml_dtypes.bfloat16
    x = np.asarray(inputs["x"], dtype=np.float32)
    Wq = np.ascontiguousarray(np.asarray(inputs["Wq"], dtype=np.float32))
    Wk = np.ascontiguousarray(np.asarray(inputs["Wk"], dtype=np.float32))
    Wv = np.ascontiguousarray(np.asarray(inputs["Wv"], dtype=np.float32))
    Wo = np.ascontiguousarray(np.asarray(inputs["Wo"], dtype=np.float32))
    bq = np.asarray(inputs["bq"], dtype=np.float32)
    bk = np.asarray(inputs["bk"], dtype=np.float32)
    bv = np.asarray(inputs["bv"], dtype=np.float32)
    bo = np.asarray(inputs["bo"], dtype=np.float32)

    xf = np.ascontiguousarray(x.reshape(NCORES * BR, S, H).astype(BF))
    cos2, sins = _rope_tables()
    cos2 = cos2.astype(BF)
    sins = sins.astype(BF)
    identm = np.eye(P, dtype=np.float32).astype(BF)
    onesb = np.ones((P, 64), dtype=BF)
    wqb, wkb, wvb, wob = (np.ascontiguousarray(w.astype(BF))
                          for w in (Wq, Wk, Wv, Wo))
    nc = _get_nc()
    in_maps = []
    for c in range(NCORES):
        in_maps.append(dict(
            xs=np.ascontiguousarray(xf[c * BR:(c + 1) * BR]),
            wq=wqb, wk=wkb, wv=wvb, wo=wob, bq=bq, bk=bk,
            cos2=cos2, sins=sins, identm=identm, onesb=onesb))
    res = run_bass_kernel_spmd(nc, in_maps, core_ids=list(range(NCORES)),
                               trace=trace)
    outs = np.concatenate([r["out"] for r in res.results], axis=0)
    out = outs.reshape(2, NCORES * BR // 2, S, H)
    out = out + (bv @ Wo + bo)
    return out.astype(np.float32), res


def kernel(**inputs):
    mask = np.asarray(inputs["mask"])
    if not np.all(mask != 0):
        return _numpy_reference(
            x=np.asarray(inputs["x"], np.float32),
            Wq=np.asarray(inputs["Wq"], np.float32),
            bq=np.asarray(inputs["bq"], np.float32),
            Wk=np.asarray(inputs["Wk"], np.float32),
            bk=np.asarray(inputs["bk"], np.float32),
            Wv=np.asarray(inputs["Wv"], np.float32),
            bv=np.asarray(inputs["bv"], np.float32),
            Wo=np.asarray(inputs["Wo"], np.float32),
            bo=np.asarray(inputs["bo"], np.float32),
            mask=mask)
    out, _ = _run(inputs, trace=False)
    return out
